# revision 1
# baseline (speedup 1.0000x reference)
"""Trainium2 Bass kernel for nn_CustomKilLayer (gnn_message_passing).

Math (from the reference):
  - prels is only consumed at row `node_index`, so the relation_pred branch
    needs a single row x = inputs_embeds[token_index[node_index]].
  - M = diag(diagonal(Ac)/deg) makes t = tprev * M diagonal, so
    t @ edges is a per-row scaling of edges by
    tdiag[i] = tprev[i,i] * Ac[i,i] / deg[i].
  - The only large memory traffic is streaming all of A (8x4096x4096 f32,
    512 MB) to form per-row sums deg[i] = sum_r w[r] * rowsum(A[r])[i].

Sharding: rows (node dim) split 512 per core across 8 cores. Each core
receives its A row-shard, the matching diagonal slices of A and tprev, its
edges row-shard, and small replicated weights. No collectives; the host
concatenates the 8 output shards.
"""

import os
import sys

import numpy as np

for _p in ("/opt/trn_rl_repo", "/root/.axon_site/_ro/trn_rl_repo"):
    if _p not in sys.path and os.path.isdir(_p):
        sys.path.append(_p)

import concourse.bass as bass
import concourse.bacc as bacc
import concourse.tile as tile
from concourse import mybir
from concourse.masks import make_identity
from concourse import bass_utils

N = 4096          # nodes
D = 256           # embedding dim
R = 8             # relations
NCORES = 8
ROWS = N // NCORES        # 512 rows per core
PT = 128                  # partition tile
TILES = ROWS // PT        # 4 row tiles per core
KB = D // PT              # 2 contraction blocks of 128 for D=256
LN_EPS = 1e-5
F32 = mybir.dt.float32

# relations whose row-sum reduction runs on the scalar (ACT) engine; the rest
# run on the vector engine (DVE). Splitting keeps both engines under the DMA
# streaming time.
ACT_RELS = (2, 5)


def _bcast_mid(ap, n):
    """Insert a stride-0 middle dim of size n into a [P, F] access pattern."""
    return bass.AP(tensor=ap.tensor, offset=ap.offset, ap=[ap.ap[0], [0, n], ap.ap[1]])


def _build_program(
    repeat=1, act_rels=ACT_RELS, astream_bufs=8, dma_engines=("sync", "scalar", "gpsimd"), gp_rels=()
):
    nc = bacc.Bacc(
        "TRN2", target_bir_lowering=False, debug=False, num_devices=NCORES
    )

    a_sh = nc.dram_tensor("a_shard", [R, ROWS, N], F32, kind="ExternalInput")
    adiag = nc.dram_tensor("adiag", [ROWS, R], F32, kind="ExternalInput")
    tpd = nc.dram_tensor("tprev_diag", [ROWS], F32, kind="ExternalInput")
    e_sh = nc.dram_tensor("edges_shard", [ROWS, D], F32, kind="ExternalInput")
    wq = nc.dram_tensor("w_q", [D, D], F32, kind="ExternalInput")
    bq = nc.dram_tensor("b_q", [1, D], F32, kind="ExternalInput")
    wv = nc.dram_tensor("w_v", [D, D], F32, kind="ExternalInput")
    bv = nc.dram_tensor("b_v", [1, D], F32, kind="ExternalInput")
    wrel = nc.dram_tensor("wrel", [1, R], F32, kind="ExternalInput")
    relst = nc.dram_tensor("rels_t", [R, D], F32, kind="ExternalInput")
    xrow = nc.dram_tensor("xrow", [1, D], F32, kind="ExternalInput")
    enidx = nc.dram_tensor("edges_nidx", [1, D], F32, kind="ExternalInput")
    out = nc.dram_tensor("out_shard", [ROWS, D], F32, kind="ExternalOutput")

    ts = bass.ts

    with tile.TileContext(nc) as tc:
        with (
            tc.tile_pool(name="consts", bufs=1) as consts,
            tc.tile_pool(name="astream", bufs=astream_bufs) as astream,
            tc.tile_pool(name="scratch", bufs=1) as scratch,
            tc.tile_pool(name="small", bufs=1) as small,
            tc.tile_pool(name="pertile", bufs=2) as pertile,
            tc.tile_pool(name="psmall", bufs=1, space="PSUM") as psmall,
            tc.tile_pool(name="pmain", bufs=2, space="PSUM") as pmain,
        ):
            # ---- constants / replicated weights ----
            ident = consts.tile([PT, PT], F32)
            make_identity(nc, ident[:])
            ones_row = consts.tile([1, PT], F32)
            nc.vector.memset(ones_row[:], 1.0)
            eps1 = consts.tile([1, 1], F32)
            nc.vector.memset(eps1[:], LN_EPS)
            eps128 = consts.tile([PT, 1], F32)
            nc.vector.memset(eps128[:], LN_EPS)

            wq_sb = consts.tile([PT, KB, D], F32)
            nc.gpsimd.dma_start(
                out=wq_sb[:], in_=wq[:, :].rearrange("(a k) n -> k a n", a=KB)
            )
            wv_sb = consts.tile([PT, KB, D], F32)
            nc.gpsimd.dma_start(
                out=wv_sb[:], in_=wv[:, :].rearrange("(a k) n -> k a n", a=KB)
            )
            xrowt_sb = consts.tile([PT, KB], F32)
            nc.gpsimd.dma_start(
                out=xrowt_sb[:], in_=xrow[0, :].rearrange("(a k) -> k a", a=KB)
            )
            bq_sb = consts.tile([1, D], F32)
            nc.gpsimd.dma_start(out=bq_sb[:], in_=bq[:, :])
            bv_sb = consts.tile([1, D], F32)
            nc.gpsimd.dma_start(out=bv_sb[:], in_=bv[:, :])
            en_sb = consts.tile([1, D], F32)
            nc.gpsimd.dma_start(out=en_sb[:], in_=enidx[:, :])
            wrel_sb = consts.tile([1, R], F32)
            nc.gpsimd.dma_start(out=wrel_sb[:], in_=wrel[:, :])
            rels_sb = consts.tile([1, R, D], F32)
            nc.gpsimd.dma_start(
                out=rels_sb[:], in_=relst[:, :].rearrange("(o r) d -> o r d", o=1)
            )
            diag_all = consts.tile([PT, TILES, R], F32)
            nc.gpsimd.dma_start(
                out=diag_all[:], in_=adiag[:, :].rearrange("(t p) r -> p t r", p=PT)
            )
            tp_all = consts.tile([PT, TILES], F32)
            nc.gpsimd.dma_start(
                out=tp_all[:], in_=tpd[:].rearrange("(t p) -> p t", p=PT)
            )

            # residual + bias row added via a K=1 matmul into PSUM
            bvres_sb = small.tile([1, D], F32)
            nc.vector.tensor_add(bvres_sb[:], bv_sb[:], en_sb[:])

            for _rep in range(repeat):
                # ---- relation_pred on one row (all on-partition-0, tiny) ----
                q_ps = psmall.tile([1, D], F32)
                for a in range(KB):
                    nc.tensor.matmul(
                        q_ps[:],
                        xrowt_sb[:, a : a + 1],
                        wq_sb[:, a, :],
                        start=(a == 0),
                        stop=(a == KB - 1),
                    )
                qb = small.tile([1, D], F32)
                nc.vector.tensor_add(qb[:], q_ps[:], bq_sb[:])
                mean1 = small.tile([1, 1], F32)
                nc.vector.reduce_sum(out=mean1[:], in_=qb[:], axis=mybir.AxisListType.X)
                nc.vector.tensor_scalar_mul(mean1[:], mean1[:], 1.0 / D)
                cent = small.tile([1, D], F32)
                nc.vector.tensor_scalar(
                    out=cent[:],
                    in0=qb[:],
                    scalar1=mean1[:],
                    scalar2=None,
                    op0=mybir.AluOpType.subtract,
                )
                sqj = small.tile([1, D], F32)
                vsum = small.tile([1, 1], F32)
                nc.scalar.activation(
                    out=sqj[:],
                    in_=cent[:],
                    func=mybir.ActivationFunctionType.Square,
                    accum_out=vsum[:],
                )
                sd1 = small.tile([1, 1], F32)
                nc.scalar.activation(
                    out=sd1[:],
                    in_=vsum[:],
                    func=mybir.ActivationFunctionType.Sqrt,
                    scale=1.0 / D,
                    bias=eps1[:],
                )
                rstd1 = small.tile([1, 1], F32)
                nc.vector.reciprocal(rstd1[:], sd1[:])
                qn = small.tile([1, D], F32)
                nc.vector.tensor_scalar(
                    out=qn[:],
                    in0=cent[:],
                    scalar1=rstd1[:],
                    scalar2=None,
                    op0=mybir.AluOpType.mult,
                )
                # logits[r] = sum_d qn[d] * relsT[r, d]
                prodj = small.tile([1, R, D], F32)
                nc.vector.tensor_tensor(
                    out=prodj[:],
                    in0=rels_sb[:],
                    in1=_bcast_mid(qn[:], R),
                    op=mybir.AluOpType.mult,
                )
                logits = small.tile([1, R], F32)
                nc.vector.reduce_sum(
                    out=logits[:], in_=prodj[:], axis=mybir.AxisListType.X
                )
                # softmax over R, then w = wrel * prels
                mx = small.tile([1, 1], F32)
                nc.vector.reduce_max(out=mx[:], in_=logits[:], axis=mybir.AxisListType.X)
                negmx = small.tile([1, 1], F32)
                nc.vector.tensor_scalar_mul(negmx[:], mx[:], -1.0)
                exps = small.tile([1, R], F32)
                sumexp = small.tile([1, 1], F32)
                nc.scalar.activation(
                    out=exps[:],
                    in_=logits[:],
                    func=mybir.ActivationFunctionType.Exp,
                    bias=negmx[:],
                    accum_out=sumexp[:],
                )
                rsum = small.tile([1, 1], F32)
                nc.vector.reciprocal(rsum[:], sumexp[:])
                w_sb = small.tile([1, R], F32)
                nc.vector.tensor_scalar(
                    out=w_sb[:],
                    in0=exps[:],
                    scalar1=rsum[:],
                    scalar2=None,
                    op0=mybir.AluOpType.mult,
                )
                nc.vector.tensor_tensor(
                    out=w_sb[:], in0=w_sb[:], in1=wrel_sb[:], op=mybir.AluOpType.mult
                )
                # broadcast w to all 128 partitions via ones[1,128].T @ w[1,R]
                wb_ps = psmall.tile([PT, R], F32)
                nc.tensor.matmul(wb_ps[:], ones_row[:], w_sb[:], start=True, stop=True)
                wb_sb = small.tile([PT, R], F32)
                nc.vector.tensor_copy(wb_sb[:], wb_ps[:])

                # ---- main loop: stream A, build row sums; per-tile epilogue ----
                act_junk = scratch.tile([PT, N], F32)
                for t in range(TILES):
                    rs_t = pertile.tile([PT, R], F32)
                    for r in range(R):
                        a_t = astream.tile([PT, N], F32)
                        eng = getattr(nc, dma_engines[(t * R + r) % len(dma_engines)])
                        eng.dma_start(out=a_t[:], in_=a_sh[r, ts(t, PT), :])
                        if r in gp_rels:
                            nc.gpsimd.tensor_reduce(
                                out=rs_t[:, r : r + 1],
                                in_=a_t[:],
                                op=mybir.AluOpType.add,
                                axis=mybir.AxisListType.X,
                            )
                        elif r in act_rels:
                            nc.scalar.activation(
                                out=act_junk[:],
                                in_=a_t[:],
                                func=mybir.ActivationFunctionType.Copy,
                                accum_out=rs_t[:, r : r + 1],
                            )
                        else:
                            nc.vector.reduce_sum(
                                out=rs_t[:, r : r + 1],
                                in_=a_t[:],
                                axis=mybir.AxisListType.X,
                            )

                    junk8 = pertile.tile([PT, R], F32)
                    deg_t = pertile.tile([PT, 1], F32)
                    nc.vector.tensor_tensor(
                        out=junk8[:], in0=rs_t[:], in1=wb_sb[:], op=mybir.AluOpType.mult
                    )
                    nc.vector.reduce_sum(
                        out=deg_t[:], in_=junk8[:], axis=mybir.AxisListType.X
                    )
                    junk8b = pertile.tile([PT, R], F32)
                    acd_t = pertile.tile([PT, 1], F32)
                    nc.vector.tensor_tensor(
                        out=junk8b[:],
                        in0=diag_all[:, t, :],
                        in1=wb_sb[:],
                        op=mybir.AluOpType.mult,
                    )
                    nc.vector.reduce_sum(
                        out=acd_t[:], in_=junk8b[:], axis=mybir.AxisListType.X
                    )
                    rdeg_t = pertile.tile([PT, 1], F32)
                    nc.vector.reciprocal(rdeg_t[:], deg_t[:])
                    tdiag_t = pertile.tile([PT, 1], F32)
                    nc.vector.tensor_scalar(
                        out=tdiag_t[:],
                        in0=acd_t[:],
                        scalar1=rdeg_t[:],
                        scalar2=tp_all[:, t : t + 1],
                        op0=mybir.AluOpType.mult,
                        op1=mybir.AluOpType.mult,
                    )

                    e_t = pertile.tile([PT, D], F32)
                    nc.sync.dma_start(out=e_t[:], in_=e_sh[ts(t, PT), :])
                    es_t = pertile.tile([PT, D], F32)
                    nc.vector.tensor_scalar(
                        out=es_t[:],
                        in0=e_t[:],
                        scalar1=tdiag_t[:],
                        scalar2=None,
                        op0=mybir.AluOpType.mult,
                    )
                    # V tile: transpose scaled edges, then (es^T)^T @ Wv + bias row
                    et_sb = pertile.tile([PT, KB, PT], F32)
                    for j in range(KB):
                        et_ps = pmain.tile([PT, PT], F32, tag="et_ps")
                        nc.tensor.transpose(et_ps[:], es_t[:, ts(j, PT)], ident[:])
                        nc.vector.tensor_copy(et_sb[:, j, :], et_ps[:])
                    v_ps = pmain.tile([PT, D], F32, tag="v_ps")
                    for j in range(KB):
                        nc.tensor.matmul(
                            v_ps[:],
                            et_sb[:, j, :],
                            wv_sb[:, j, :],
                            start=(j == 0),
                            stop=False,
                        )
                    nc.tensor.matmul(
                        v_ps[:], ones_row[:], bvres_sb[:], start=False, stop=True
                    )

                    # layernorm rows of v_ps
                    stats = pertile.tile([PT, 6], F32)
                    nc.vector.bn_stats(out=stats[:], in_=v_ps[:])
                    mv = pertile.tile([PT, 2], F32)
                    nc.vector.bn_aggr(out=mv[:], in_=stats[:])
                    sd_t = pertile.tile([PT, 1], F32)
                    nc.scalar.activation(
                        out=sd_t[:],
                        in_=mv[:, 1:2],
                        func=mybir.ActivationFunctionType.Sqrt,
                        bias=eps128[:],
                    )
                    rstd_t = pertile.tile([PT, 1], F32)
                    nc.vector.reciprocal(rstd_t[:], sd_t[:])
                    out_t = pertile.tile([PT, D], F32)
                    nc.vector.tensor_scalar(
                        out=out_t[:],
                        in0=v_ps[:],
                        scalar1=mv[:, 0:1],
                        scalar2=rstd_t[:],
                        op0=mybir.AluOpType.subtract,
                        op1=mybir.AluOpType.mult,
                    )
                    nc.sync.dma_start(out=out[ts(t, PT), :], in_=out_t[:])

    nc.compile()
    return nc


_NC_CACHE = None


def _get_nc():
    global _NC_CACHE
    if _NC_CACHE is None:
        _NC_CACHE = _build_program()
    return _NC_CACHE


def _make_in_maps(inputs):
    f32 = lambda x: np.ascontiguousarray(np.asarray(x), dtype=np.float32)
    inputs_embeds = f32(inputs["inputs_embeds"])
    token_index = np.asarray(inputs["token_index"])
    node_index = int(np.asarray(inputs["node_index"]))
    edges = f32(inputs["edges"])
    A = np.asarray(inputs["A"], dtype=np.float32)
    rels = f32(inputs["rels"])
    wrel = f32(inputs["wrel"]).reshape(1, R)
    W_q = f32(inputs["W_q"])
    b_q = f32(inputs["b_q"]).reshape(1, D)
    W_v = f32(inputs["W_v"])
    b_v = f32(inputs["b_v"]).reshape(1, D)
    tprev = np.asarray(inputs["tprev"], dtype=np.float32)

    row = int(token_index[node_index])
    xrow = np.ascontiguousarray(inputs_embeds[row]).reshape(1, D)
    enidx = np.ascontiguousarray(edges[node_index]).reshape(1, D)
    relst = np.ascontiguousarray(rels.T)          # [R, D]
    tprev_diag = np.ascontiguousarray(np.diagonal(tprev))  # [N]
    a_diag = np.ascontiguousarray(
        np.transpose(np.diagonal(A, axis1=1, axis2=2))
    )  # [N, R]

    in_maps = []
    for c in range(NCORES):
        lo, hi = c * ROWS, (c + 1) * ROWS
        in_maps.append(
            {
                "a_shard": np.ascontiguousarray(A[:, lo:hi, :]),
                "adiag": np.ascontiguousarray(a_diag[lo:hi]),
                "tprev_diag": np.ascontiguousarray(tprev_diag[lo:hi]),
                "edges_shard": np.ascontiguousarray(edges[lo:hi]),
                "w_q": W_q,
                "b_q": b_q,
                "w_v": W_v,
                "b_v": b_v,
                "wrel": wrel,
                "rels_t": relst,
                "xrow": xrow,
                "edges_nidx": enidx,
            }
        )
    return in_maps


def run(trace=False, **inputs):
    """Run the kernel; returns (full_output, BassKernelResults)."""
    nc = _get_nc()
    in_maps = _make_in_maps(inputs)
    res = bass_utils.run_bass_kernel_spmd(
        nc, in_maps, core_ids=list(range(NCORES)), trace=trace
    )
    outp = np.concatenate(
        [np.asarray(res.results[c]["out_shard"]) for c in range(NCORES)], axis=0
    )
    return outp.astype(np.float32), res


def kernel(**inputs):
    outp, _ = run(trace=False, **inputs)
    return outp

